# revision 1
# baseline (speedup 1.0000x reference)
"""DVH global loss (histogram binning) Trainium2 kernel.

Strategy: 8 cores, data-parallel over (batch, voxel-half): core = 2*b + h.
Each core computes a joint 16x32 (q, r) histogram of the dose-bin index
j = floor-ish(d * 499/75) (j = searchsorted(linspace(0,75,500), d*m,
'right') - 1 up to ulp-boundary noise), via exact fp32 magic-number
rounding chains split across DVE and ACT; bf16 one-hot expansion on DVE;
PE accumulates outer(A_col, B_col) over all voxel columns into PSUM[16,32].
Masked voxels are pushed past bin 4000 so their q >= 125 misses the 16-wide
q-one-hot entirely (counts only unmasked voxels). Host combines: signed
hist e = H_pred - H_gt per batch, reverse-cumsum -> DVH count differences,
MSE over (batch, bin) with per-batch denom = sum(mask) + 1e-6. Histogram
counts are integer-exact in fp32.

A post-Tile pass legalizes semaphore waits: trn2 engine instructions have
very few sync-wait slots (TensorTensor/DMA structs fit one), so redundant
same-engine waits are dropped (engine queues are strict in-order FIFO) and
excess waits move onto same-engine NOPs inserted before the instruction.
"""

import sys
from contextlib import ExitStack

if "/opt/trn_rl_repo" not in sys.path:
    sys.path.insert(0, "/opt/trn_rl_repo")

import numpy as np

import concourse.bass as bass
import concourse.tile as tile
from concourse import mybir
from concourse.bass_utils import run_bass_kernel_spmd

F32 = mybir.dt.float32
BF16 = mybir.dt.bfloat16

C1 = 499.0 / 75.0
GUARD = 0.4998
U2_S1 = -4000.0 / C1  # * m
U2_S2 = (4000.0 - GUARD) / C1  # + const


# trn2 engine instructions have very few sync-wait slots (TT has one). Tile
# emits redundant same-engine waits and multi-waits that walrus rejects.
# Legalize: drop own-engine-sem waits on in-order compute engines, then move
# excess waits onto earlier same-engine instructions with free slots.
_ENGINE_SEM_PREFIX = {
    mybir.EngineType.DVE: "DVE_",
    mybir.EngineType.Activation: "Activation_",
    mybir.EngineType.Pool: "Pool_",
}
_COMPUTE_ENGINES = (
    mybir.EngineType.DVE,
    mybir.EngineType.Activation,
    mybir.EngineType.Pool,
    mybir.EngineType.PE,
)


_EXEMPT_TYPES = (
    "InstCall",
    "InstUnconditionalBranch",
    "InstRegisterMove",
    "InstISA",
    "InstNoOp",
)

_SELF_DROP_TYPES = (
    "InstTensorTensor",
    "InstTensorScalarPtr",
    "InstTensorReduce",
    "InstActivation",
    "InstMemset",
    "InstTensorCopy",
)


def legalize_sync_waits(nc, max_waits=1):
    """trn2 engine instructions have very few sync-wait slots (TT and DMA
    structs have one). Drop redundant same-engine waits on in-order compute
    engines, then split remaining excess waits onto same-engine NOPs
    inserted immediately before the instruction."""
    eng_map = {
        mybir.EngineType.DVE: nc.vector,
        mybir.EngineType.Activation: nc.scalar,
        mybir.EngineType.Pool: nc.gpsimd,
        mybir.EngineType.PE: nc.tensor,
        mybir.EngineType.SP: nc.sync,
    }
    for fn in nc.m.functions:
        blocks = list(fn.blocks)
        for blk in blocks:
            insts = blk.instructions
            work = []
            for i, ins in enumerate(insts):
                tname = type(ins).__name__
                if tname in _EXEMPT_TYPES:
                    continue
                si = ins.sync_info
                if si is None:
                    continue
                waits = list(si.on_wait)
                eng = ins.engine
                pref = _ENGINE_SEM_PREFIX.get(eng)
                if pref is not None and tname in _SELF_DROP_TYPES:
                    waits = [
                        w for w in waits
                        if not (w.ant_name or "").startswith(pref)
                    ]
                if len(waits) == len(si.on_wait) and len(waits) <= max_waits:
                    continue
                work.append((i, ins, waits))
            for i, ins, waits in reversed(work):
                si = ins.sync_info
                keep, excess = waits[:max_waits], waits[max_waits:]
                ins.sync_info = mybir.SyncInfo(
                    on_wait=keep, on_update=si.on_update
                )
                eng_iface = eng_map[ins.engine]
                for w in reversed(excess):
                    bi = eng_iface.nop(nofuse=True)
                    mi = bi.ins
                    for b2 in fn.blocks:
                        L = b2.instructions
                        for k in range(len(L) - 1, -1, -1):
                            if L[k] is mi or L[k].name == mi.name:
                                del L[k]
                                break
                        else:
                            continue
                        break
                    mi.sync_info = mybir.SyncInfo(on_wait=[w], on_update=[])
                    blk.instructions.insert(i, mi)


def build_kernel(P=128, FPP=8192, F=256, QW=16, RW=32, debug=False,
                 ah_on_pool=False, bh_split=0):
    assert FPP % F == 0
    nchunks = FPP // F
    nc = bass.Bass()

    d_p_ext = nc.declare_dram_parameter("d_pred", [P, FPP], F32, isOutput=False)
    d_g_ext = nc.declare_dram_parameter("d_gt", [P, FPP], F32, isOutput=False)
    m_ext = nc.declare_dram_parameter("mask", [P, FPP], F32, isOutput=False)
    hist_p_ext = nc.declare_dram_parameter("hist_p", [P, RW], F32, isOutput=True)
    hist_g_ext = nc.declare_dram_parameter("hist_g", [P, RW], F32, isOutput=True)
    msum_ext = nc.declare_dram_parameter("msum", [P, nchunks], F32, isOutput=True)
    if debug:
        dbg_q = nc.declare_dram_parameter("dbg_q", [P, F], BF16, isOutput=True)
        dbg_r = nc.declare_dram_parameter("dbg_r", [P, F], BF16, isOutput=True)
        dbg_t = nc.declare_dram_parameter("dbg_t", [P, F], F32, isOutput=True)

    with tile.TileContext(nc) as tc, ExitStack() as ctx:
        singles = ctx.enter_context(tc.tile_pool(name="singles", bufs=1))
        ins = ctx.enter_context(tc.tile_pool(name="ins", bufs=3))
        mids = ctx.enter_context(tc.tile_pool(name="mids", bufs=2))
        hots = ctx.enter_context(tc.tile_pool(name="hots", bufs=2))
        psums = ctx.enter_context(
            tc.tile_pool(name="psums", bufs=2, space=bass.MemorySpace.PSUM)
        )

        # constant one-hot comparison patterns (DVE-built so later DVE
        # readers need no cross-engine wait)
        iota_a = singles.tile([P, QW, F], BF16)
        for w in range(QW):
            nc.vector.memset(iota_a[:, w, :], float(w))
        iota_b = singles.tile([P, RW, F], BF16)
        for w in range(RW):
            nc.vector.memset(iota_b[:, w, :], float(w))

        acc_p = singles.tile([P, RW], F32)
        acc_g = singles.tile([P, RW], F32)
        nc.vector.memset(acc_p, 0.0)
        nc.vector.memset(acc_g, 0.0)
        msum = singles.tile([P, nchunks], F32)

        for c in range(nchunks):
            sl = slice(c * F, (c + 1) * F)
            d_p = ins.tile([P, F], F32, tag="d_p")
            d_g = ins.tile([P, F], F32, tag="d_g")
            m = ins.tile([P, F], F32, tag="m")
            nc.sync.dma_start(out=d_p, in_=d_p_ext[:, sl])
            nc.sync.dma_start(out=d_g, in_=d_g_ext[:, sl])
            nc.sync.dma_start(out=m, in_=m_ext[:, sl])

            # u2 = (4000*(1-m) - guard)/C1
            u0 = mids.tile([P, F], F32, tag="u0")
            nc.vector.tensor_scalar(
                out=u0, in0=m, scalar1=U2_S1, scalar2=None,
                op0=mybir.AluOpType.mult,
            )
            u = mids.tile([P, F], F32, tag="u")
            nc.vector.tensor_scalar(
                out=u, in0=u0, scalar1=U2_S2, scalar2=None,
                op0=mybir.AluOpType.add,
            )
            nc.vector.tensor_reduce(
                out=msum[:, c : c + 1], in_=m, axis=mybir.AxisListType.X,
                op=mybir.AluOpType.add,
            )

            for which, d_t, accum in (("p", d_p, acc_p), ("g", d_g, acc_g)):
                x2 = mids.tile([P, F], F32, tag="x2")
                nc.vector.tensor_tensor(
                    out=x2, in0=d_t, in1=u, op=mybir.AluOpType.add
                )
                # ---- ACT chain: only the first op waits on DVE ----
                t = mids.tile([P, F], F32, tag="t")
                nc.scalar.activation(
                    out=t, in_=x2, func=mybir.ActivationFunctionType.Copy,
                    bias=12582912.0, scale=C1,
                )
                f1 = mids.tile([P, F], F32, tag="f1")
                nc.scalar.activation(
                    out=f1, in_=t, func=mybir.ActivationFunctionType.Copy,
                    bias=-393216.0, scale=0.03125,
                )
                f2 = mids.tile([P, F], F32, tag="f2")
                nc.scalar.activation(
                    out=f2, in_=f1, func=mybir.ActivationFunctionType.Copy,
                    bias=-0.484375, scale=1.0,
                )
                qm = mids.tile([P, F], F32, tag="qm")
                nc.scalar.activation(
                    out=qm, in_=f2, func=mybir.ActivationFunctionType.Copy,
                    bias=12582912.0, scale=1.0,
                )
                q_bf = mids.tile([P, F], BF16, tag="q_bf")
                nc.scalar.activation(
                    out=q_bf, in_=qm, func=mybir.ActivationFunctionType.Copy,
                    bias=-12582912.0, scale=1.0,
                )
                v = mids.tile([P, F], F32, tag="v")
                nc.scalar.activation(
                    out=v, in_=qm, func=mybir.ActivationFunctionType.Copy,
                    bias=-390070272.0, scale=32.0,
                )
                # ---- back to DVE ----
                r_bf = mids.tile([P, F], BF16, tag="r_bf")
                nc.vector.tensor_tensor(
                    out=r_bf, in0=t, in1=v, op=mybir.AluOpType.subtract
                )
                ah = hots.tile([P, QW, F], BF16, tag="ah")
                ah_eng = nc.gpsimd if ah_on_pool else nc.vector
                ah_eng.tensor_tensor(
                    out=ah, in0=q_bf[:, None, :].broadcast_to([P, QW, F]),
                    in1=iota_a, op=mybir.AluOpType.is_equal,
                )
                bh = hots.tile([P, RW, F], BF16, tag="bh")
                if bh_split > 0:
                    k = bh_split
                    nc.gpsimd.tensor_tensor(
                        out=bh[:, :k, :],
                        in0=r_bf[:, None, :].broadcast_to([P, k, F]),
                        in1=iota_b[:, :k, :], op=mybir.AluOpType.is_equal,
                    )
                    nc.vector.tensor_tensor(
                        out=bh[:, k:, :],
                        in0=r_bf[:, None, :].broadcast_to([P, RW - k, F]),
                        in1=iota_b[:, k:, :], op=mybir.AluOpType.is_equal,
                    )
                else:
                    nc.vector.tensor_tensor(
                        out=bh, in0=r_bf[:, None, :].broadcast_to([P, RW, F]),
                        in1=iota_b, op=mybir.AluOpType.is_equal,
                    )

                if debug and c == 0 and which == "p":
                    nc.sync.dma_start(out=dbg_q[:], in_=q_bf)
                    nc.sync.dma_start(out=dbg_r[:], in_=r_bf)
                    nc.sync.dma_start(out=dbg_t[:], in_=t)

                # 3-way PE column-group concurrency: column f accumulates
                # into PSUM partition block 32*(f%3); host sums the 3 blocks.
                # (AP base_partition 96 is not supported, else 4-way.)
                ps = psums.tile([P, RW], F32, tag="ps")
                for f in range(F):
                    j = f % 3
                    nc.tensor.matmul(
                        ps[32 * j : 32 * j + QW, :], ah[:, :, f], bh[:, :, f],
                        start=(f < 3), stop=(f >= F - 3),
                    )
                for j in range(3):
                    sl32 = slice(32 * j, 32 * j + QW)
                    nc.vector.tensor_tensor(
                        out=accum[sl32, :], in0=accum[sl32, :],
                        in1=ps[sl32, :], op=mybir.AluOpType.add,
                    )

        nc.sync.dma_start(out=hist_p_ext[:], in_=acc_p)
        nc.sync.dma_start(out=hist_g_ext[:], in_=acc_g)
        nc.sync.dma_start(out=msum_ext[:], in_=msum)

    legalize_sync_waits(nc)
    return nc



NCORES = 8
P = 128
FPP = 8192  # voxels per partition per core (half a 128^3 volume / 128)
QW, RW = 16, 32

_CACHE = {}


def _get_nc():
    if "nc" not in _CACHE:
        _CACHE["nc"] = build_kernel(P=P, FPP=FPP, F=256, QW=QW, RW=RW)
    return _CACHE["nc"]


def run_device(d_pred, d_gt, mask, trace=False, tmpdir=None):
    """Run the SPMD kernel; returns (results_list, exec_time_ns)."""
    B = d_pred.shape[0]
    V = int(np.prod(d_pred.shape[1:]))
    dp = np.ascontiguousarray(d_pred, dtype=np.float32).reshape(B, V)
    dg = np.ascontiguousarray(d_gt, dtype=np.float32).reshape(B, V)
    mm = np.ascontiguousarray(mask, dtype=np.float32).reshape(B, V)
    half = V // 2
    in_maps = []
    for core in range(NCORES):
        b, h = divmod(core, 2)
        sl = slice(h * half, (h + 1) * half)
        in_maps.append(
            {
                "d_pred": dp[b, sl].reshape(P, FPP),
                "d_gt": dg[b, sl].reshape(P, FPP),
                "mask": mm[b, sl].reshape(P, FPP),
            }
        )
    res = run_bass_kernel_spmd(
        _get_nc(), in_maps, list(range(NCORES)), trace=trace, tmpdir=tmpdir
    )
    return res.results, res.exec_time_ns


def kernel(d_pred, d_gt, mask):
    results, _ = run_device(d_pred, d_gt, mask)
    B = d_pred.shape[0]
    loss = 0.0
    for b in range(B):
        e = np.zeros((QW, RW), np.float64)
        msum = 0.0
        for h in range(2):
            r = results[2 * b + h]
            hp = r["hist_p"].astype(np.float64)
            hg = r["hist_g"].astype(np.float64)
            for j in range(3):
                e += hp[32 * j : 32 * j + QW, :] - hg[32 * j : 32 * j + QW, :]
            msum += float(r["msum"].sum(dtype=np.float64))
        ed = e.reshape(QW * RW)[:500]
        T = np.cumsum(ed[::-1])[::-1]
        denom = msum + 1e-6
        loss += float(np.sum((T / denom) ** 2))
    loss /= B * 500
    return np.float32(loss)



# revision 4
# speedup vs baseline: 1.1706x; 1.1706x over previous
"""DVH global loss (histogram binning) Trainium2 kernel.

8 cores, data-parallel over (batch, voxel-half): core = 2*b + h.
Each core histograms 1M voxels x 2 tensors (pred, gt) into 512 bins
j = 32*q + r via a PE pairing of per-voxel encodings:

  out[wr*4+g, wq*4+g] += sum_vox enc_r(r_n; wr) * onehot(q_n; wq)

- moving side (q, 16 slots): exact one-hot, built on DVE via per-slot
  tensor_scalar is_equal over bf16 q values.
- stationary side (r, 32 slots): mixed encoding -- one-hot slots on DVE,
  |r - w| (Abs) slots on the ACT engine (1 op/slot; values <= 31 exact in
  bf16).  The host inverts the known 32x32 encoding matrix (delta rows +
  abs rows) in float64; counts are integer-exact.
- masked voxels: x2 = d + u with u = PUSH*(1-m) - GUARD/C1 pushes their
  q out of [0,16), so the q one-hot zeroes their contribution.
- G=8 junk-block matmuls: stationary slab [128, 16*8] (ah4[:, i, :, :]
  contiguous), moving [128, 32*8] sliced from the natural-layout bh
  (matmul moving APs may be multi-dim); only diagonal group blocks
  g==g' are decoded.  One PSUM accumulation chain per chunk keeps every
  fp32 PSUM cell integer-exact (<= 65536 voxels x 31); chains are
  drained (DVE copy) into an SBUF stack, DMA'd out, and summed on the
  host in float64.

A post-Tile pass legalizes semaphore waits (trn2 wait-slot limits).
"""

import sys
from contextlib import ExitStack

if "/opt/trn_rl_repo" not in sys.path:
    sys.path.insert(0, "/opt/trn_rl_repo")

import numpy as np

import concourse.bass as bass
import concourse.tile as tile
from concourse import mybir
from concourse.bass_utils import run_bass_kernel_spmd

F32 = mybir.dt.float32
BF16 = mybir.dt.bfloat16

NUM_BINS = 500
DOSE_MAX = 75.0
C1 = 499.0 / DOSE_MAX
GUARD = 0.4998
MAGIC = 12582912.0  # 1.5 * 2**23
JSTAR = 505  # masked voxels collapse to this bin
QSTAR, RSTAR = JSTAR // 32, JSTAR % 32
XSTAR = (JSTAR + 0.3) / C1  # maps to JSTAR with rounding margin

QW, RW, G = 16, 32, 8

# r-side slot encodings: "d" = delta (one-hot, DVE), "a" = abs (ACT)
# abs slots spread across the range for conditioning.
ACT_R_SET = (0, 2, 5, 8, 11, 14, 17, 20, 23, 26, 29)
ONES_R = 31  # constant all-ones column (cheap memset, row of ones in K)
DVE_R_SET = tuple(
    w for w in range(RW) if w not in ACT_R_SET and w != ONES_R
)

# ---- sync-wait legalization (same as baseline) ----
_ENGINE_SEM_PREFIX = {
    mybir.EngineType.DVE: "DVE_",
    mybir.EngineType.Activation: "Activation_",
    mybir.EngineType.Pool: "Pool_",
}
_EXEMPT_TYPES = (
    "InstCall",
    "InstUnconditionalBranch",
    "InstRegisterMove",
    "InstISA",
    "InstNoOp",
)
_SELF_DROP_TYPES = (
    "InstTensorTensor",
    "InstTensorScalarPtr",
    "InstTensorReduce",
    "InstActivation",
    "InstMemset",
    "InstTensorCopy",
)


def legalize_sync_waits(nc, max_waits=1):
    eng_map = {
        mybir.EngineType.DVE: nc.vector,
        mybir.EngineType.Activation: nc.scalar,
        mybir.EngineType.Pool: nc.gpsimd,
        mybir.EngineType.PE: nc.tensor,
        mybir.EngineType.SP: nc.sync,
    }
    for fn in nc.m.functions:
        blocks = list(fn.blocks)
        for blk in blocks:
            insts = blk.instructions
            work = []
            for i, ins in enumerate(insts):
                tname = type(ins).__name__
                if tname in _EXEMPT_TYPES:
                    continue
                si = ins.sync_info
                if si is None:
                    continue
                waits = list(si.on_wait)
                eng = ins.engine
                pref = _ENGINE_SEM_PREFIX.get(eng)
                if pref is not None and tname in _SELF_DROP_TYPES:
                    waits = [
                        w for w in waits
                        if not (w.ant_name or "").startswith(pref)
                    ]
                if len(waits) == len(si.on_wait) and len(waits) <= max_waits:
                    continue
                work.append((i, ins, waits))
            for i, ins, waits in reversed(work):
                si = ins.sync_info
                keep, excess = waits[:max_waits], waits[max_waits:]
                ins.sync_info = mybir.SyncInfo(
                    on_wait=keep, on_update=si.on_update
                )
                eng_iface = eng_map[ins.engine]
                for w in reversed(excess):
                    bi = eng_iface.nop(nofuse=True)
                    mi = bi.ins
                    for b2 in fn.blocks:
                        L = b2.instructions
                        for k in range(len(L) - 1, -1, -1):
                            if L[k] is mi or L[k].name == mi.name:
                                del L[k]
                                break
                        else:
                            continue
                        break
                    mi.sync_info = mybir.SyncInfo(on_wait=[w], on_update=[])
                    blk.instructions.insert(i, mi)


def build_kernel(P=128, FPP=8192, F=512, chain_chunks=1):
    assert FPP % F == 0
    nchunks = FPP // F
    NI = F // G
    nchains = nchunks // chain_chunks
    nc = bass.Bass()

    d_p_ext = nc.declare_dram_parameter("d_pred", [P, FPP], F32, isOutput=False)
    d_g_ext = nc.declare_dram_parameter("d_gt", [P, FPP], F32, isOutput=False)
    m_ext = nc.declare_dram_parameter("mask", [P, FPP], F32, isOutput=False)
    hp_ext = nc.declare_dram_parameter("hp", [P, nchains * 256], F32, isOutput=True)
    hg_ext = nc.declare_dram_parameter("hg", [P, nchains * 256], F32, isOutput=True)

    with tile.TileContext(nc) as tc, ExitStack() as ctx:
        singles = ctx.enter_context(tc.tile_pool(name="singles", bufs=1))
        ins = ctx.enter_context(tc.tile_pool(name="ins", bufs=3))
        mids = ctx.enter_context(tc.tile_pool(name="mids", bufs=2))
        hots = ctx.enter_context(tc.tile_pool(name="hots", bufs=2))
        psums = ctx.enter_context(
            tc.tile_pool(name="psums", bufs=1, space=bass.MemorySpace.PSUM)
        )

        # per-ACT-slot bias tiles (-w) for Abs encoding
        bias_t = {}
        for w in ACT_R_SET:
            bt = singles.tile([P, 1], F32, tag=f"bias{w}")
            nc.vector.memset(bt, float(-w))
            bias_t[w] = bt

        drains_p = singles.tile([P, nchains, 256], F32)
        drains_g = singles.tile([P, nchains, 256], F32)

        ps_p = psums.tile([P, 256], F32)
        ps_g = psums.tile([P, 256], F32)
        ps_map = {"p": ps_p, "g": ps_g}
        dr_map = {"p": drains_p, "g": drains_g}

        PUSH = 640.0 / C1  # masked voxels -> j ~ [640, 1140): q >= 20
        U_S1 = -PUSH
        U_S2 = PUSH - GUARD / C1

        for c in range(nchunks):
            sl = slice(c * F, (c + 1) * F)
            d_p = ins.tile([P, F], F32, tag="d_p")
            d_g = ins.tile([P, F], F32, tag="d_g")
            m = ins.tile([P, F], F32, tag="m")
            nc.sync.dma_start(out=d_p, in_=d_p_ext[:, sl])
            nc.sync.dma_start(out=d_g, in_=d_g_ext[:, sl])
            nc.sync.dma_start(out=m, in_=m_ext[:, sl])

            # u = m * (-PUSH) + (PUSH - GUARD/C1): pushes masked q out of
            # [0,16) so the q one-hot zeroes their contribution entirely.
            wv = mids.tile([P, F], F32, tag="wv")
            nc.vector.tensor_scalar(
                out=wv, in0=m, scalar1=U_S1, scalar2=U_S2,
                op0=mybir.AluOpType.mult, op1=mybir.AluOpType.add,
            )
            first_chain_chunk = c % chain_chunks == 0
            last_chain_chunk = c % chain_chunks == chain_chunks - 1
            chain_idx = c // chain_chunks

            for which, d_t in (("p", d_p), ("g", d_g)):
                ps = ps_map[which]
                # x2 = d + u
                x2 = mids.tile([P, F], F32, tag="x2")
                nc.vector.tensor_tensor(
                    out=x2, in0=d_t, in1=wv, op=mybir.AluOpType.add
                )
                # ACT chain: t = rnd(C1*x2 + MAGIC) = MAGIC + j
                t = mids.tile([P, F], F32, tag="t")
                nc.scalar.activation(
                    out=t, in_=x2, func=mybir.ActivationFunctionType.Copy,
                    bias=MAGIC, scale=C1,
                )
                # f2 = t/32 - 393216.484375 = j/32 - 0.484375  (exact)
                f2 = mids.tile([P, F], F32, tag="f2")
                nc.scalar.activation(
                    out=f2, in_=t, func=mybir.ActivationFunctionType.Copy,
                    bias=-393216.484375, scale=0.03125,
                )
                # qm = rnd(f2 + MAGIC) = MAGIC + q
                qm = mids.tile([P, F], F32, tag="qm")
                nc.scalar.activation(
                    out=qm, in_=f2, func=mybir.ActivationFunctionType.Copy,
                    bias=MAGIC, scale=1.0,
                )
                # q_bf = qm - MAGIC (bf16, exact: q in [0,15])
                q_bf = mids.tile([P, F], BF16, tag="q_bf")
                nc.scalar.activation(
                    out=q_bf, in_=qm, func=mybir.ActivationFunctionType.Copy,
                    bias=-MAGIC, scale=1.0,
                )
                # v = 32*qm - 32*MAGIC + MAGIC = MAGIC + 32q   (DVE)
                v = mids.tile([P, F], F32, tag="v")
                nc.vector.tensor_scalar(
                    out=v, in0=qm, scalar1=32.0, scalar2=-390070272.0,
                    op0=mybir.AluOpType.mult, op1=mybir.AluOpType.add,
                )
                # r_bf = t - v (bf16, exact: r in [0,31])
                r_bf = mids.tile([P, F], BF16, tag="r_bf")
                nc.vector.tensor_tensor(
                    out=r_bf, in0=t, in1=v, op=mybir.AluOpType.subtract
                )

                q4 = q_bf.rearrange("p (a b) -> p a b", b=G)

                # stationary side: ah4[p, i, wq, g] = (q == wq)  (slab layout)
                ah4 = hots.tile([P, NI, QW, G], BF16, tag="ah4")
                for w in range(QW):
                    nc.vector.tensor_scalar(
                        out=ah4[:, :, w, :], in0=q4, scalar1=float(w),
                        scalar2=None, op0=mybir.AluOpType.is_equal,
                    )
                # moving side: bh[p, wr, f], natural layout (contiguous
                # builder writes; matmul moving AP may be multi-dim)
                bh = hots.tile([P, RW, F], BF16, tag="bh")
                nc.vector.memset(bh[:, ONES_R, :], 1.0)
                for w in DVE_R_SET:
                    nc.vector.tensor_scalar(
                        out=bh[:, w, :], in0=r_bf, scalar1=float(w),
                        scalar2=None, op0=mybir.AluOpType.is_equal,
                    )
                for w in ACT_R_SET:
                    nc.scalar.activation(
                        out=bh[:, w, :], in_=r_bf,
                        func=mybir.ActivationFunctionType.Abs,
                        bias=bias_t[w][:, 0:1], scale=1.0,
                    )

                for i in range(NI):
                    nc.tensor.matmul(
                        ps, ah4[:, i, :, :], bh[:, :, i * G : (i + 1) * G],
                        start=(first_chain_chunk and i == 0),
                        stop=(last_chain_chunk and i == NI - 1),
                    )
                if last_chain_chunk:
                    nc.vector.tensor_copy(
                        out=dr_map[which][:, chain_idx, :], in_=ps
                    )

        nc.sync.dma_start(
            out=hp_ext[:], in_=drains_p.rearrange("p a b -> p (a b)")
        )
        nc.sync.dma_start(
            out=hg_ext[:], in_=drains_g.rearrange("p a b -> p (a b)")
        )

    legalize_sync_waits(nc)
    return nc


NCORES = 8
P = 128
FPP = 8192
F = 512
NCHUNKS = FPP // F
CHAIN_CHUNKS = 1
NCHAINS = NCHUNKS // CHAIN_CHUNKS

_CACHE = {}


def _get_nc():
    if "nc" not in _CACHE:
        _CACHE["nc"] = build_kernel(P=P, FPP=FPP, F=F, chain_chunks=CHAIN_CHUNKS)
    return _CACHE["nc"]


def _encoding_matrix():
    """K[w, r]: delta rows for DVE slots, |r - w| rows for ACT slots."""
    K = np.zeros((RW, RW), np.float64)
    for w in range(RW):
        if w in ACT_R_SET:
            K[w, :] = np.abs(np.arange(RW) - w)
        elif w == ONES_R:
            K[w, :] = 1.0
        else:
            K[w, w] = 1.0
    return K


_KINV = np.linalg.inv(_encoding_matrix())


def _enc_vec(r):
    """Encoding response column for a voxel at r: g[w] = K[w, r]."""
    g = np.zeros(RW, np.float64)
    for w in range(RW):
        if w in ACT_R_SET:
            g[w] = abs(r - w)
        elif w == ONES_R:
            g[w] = 1.0
        else:
            g[w] = 1.0 if w == r else 0.0
    return g


def run_device(d_pred, d_gt, mask, trace=False, tmpdir=None):
    B = d_pred.shape[0]
    V = int(np.prod(d_pred.shape[1:]))
    dp = np.ascontiguousarray(d_pred, dtype=np.float32).reshape(B, V)
    dg = np.ascontiguousarray(d_gt, dtype=np.float32).reshape(B, V)
    mm = np.ascontiguousarray(mask, dtype=np.float32).reshape(B, V)
    half = V // 2
    in_maps = []
    for core in range(NCORES):
        b, h = divmod(core, 2)
        sl = slice(h * half, (h + 1) * half)
        in_maps.append(
            {
                "d_pred": dp[b, sl].reshape(P, FPP),
                "d_gt": dg[b, sl].reshape(P, FPP),
                "mask": mm[b, sl].reshape(P, FPP),
            }
        )
    res = run_bass_kernel_spmd(
        _get_nc(), in_maps, list(range(NCORES)), trace=trace, tmpdir=tmpdir
    )
    return res.results, res.exec_time_ns


def _decode_hist(hraw, n_masked):
    """hraw: [P, NCHAINS*256] f32 -> histogram [512] (float64)."""
    M = hraw.astype(np.float64).reshape(P, NCHAINS, 256).sum(axis=1)
    # M[wq*8+g, wr*8+g'] ; diagonal blocks g==g'
    M4 = M.reshape(QW, G, RW, G)
    Mfull = np.zeros((RW, QW), np.float64)
    for g in range(G):
        Mfull += M4[:, g, :, g].T
    # invert r-encoding per q
    J = _KINV @ Mfull  # [r, q]
    hist = np.zeros(512, np.float64)
    for q in range(QW):
        hist[32 * q : 32 * q + 32] = J[:, q]
    return hist


def kernel(d_pred, d_gt, mask):
    results, _ = run_device(d_pred, d_gt, mask)
    B = d_pred.shape[0]
    V = int(np.prod(d_pred.shape[1:]))
    mm = np.asarray(mask, dtype=np.float64).reshape(B, V)
    half = V // 2
    core_msums = [
        float(mm[c // 2, (c % 2) * half : (c % 2 + 1) * half].sum())
        for c in range(NCORES)
    ]
    loss = 0.0
    for b in range(B):
        e = np.zeros(512, np.float64)
        msum = 0.0
        for h in range(2):
            r = results[2 * b + h]
            core_msum = core_msums[2 * b + h]
            n_masked = P * FPP - core_msum
            hp = _decode_hist(r["hp"], n_masked)
            hg = _decode_hist(r["hg"], n_masked)
            e += hp - hg
            msum += core_msum
        ed = e[:500]
        T = np.cumsum(ed[::-1])[::-1]
        denom = msum + 1e-6
        loss += float(np.sum((T / denom) ** 2))
    loss /= B * 500
    return np.float32(loss)


# revision 6
# speedup vs baseline: 1.2183x; 1.0407x over previous
"""DVH global loss (histogram binning) Trainium2 kernel.

8 cores, data-parallel over (batch, voxel-half): core = 2*b + h.
Each core histograms 1M voxels x 2 tensors (pred, gt) into 512 bins
j = 32*q + r via a PE pairing of per-voxel encodings:

  out[wr*4+g, wq*4+g] += sum_vox enc_r(r_n; wr) * onehot(q_n; wq)

- moving side (q, 16 slots): exact one-hot, built on DVE via per-slot
  tensor_scalar is_equal over bf16 q values.
- stationary side (r, 32 slots): mixed encoding -- one-hot slots on DVE,
  |r - w| (Abs) slots on the ACT engine (1 op/slot; values <= 31 exact in
  bf16).  The host inverts the known 32x32 encoding matrix (delta rows +
  abs rows) in float64; counts are integer-exact.
- masked voxels: x2 = d + u with u = PUSH*(1-m) - GUARD/C1 pushes their
  q out of [0,16), so the q one-hot zeroes their contribution.
- G=8 junk-block matmuls: stationary slab [128, 16*8] (ah4[:, i, :, :]
  contiguous), moving [128, 32*8] sliced from the natural-layout bh
  (matmul moving APs may be multi-dim); only diagonal group blocks
  g==g' are decoded.  One PSUM accumulation chain per chunk keeps every
  fp32 PSUM cell integer-exact (<= 65536 voxels x 31); chains are
  drained (DVE copy) into an SBUF stack, DMA'd out, and summed on the
  host in float64.

A post-Tile pass legalizes semaphore waits (trn2 wait-slot limits).
"""

import sys
from contextlib import ExitStack

if "/opt/trn_rl_repo" not in sys.path:
    sys.path.insert(0, "/opt/trn_rl_repo")

import numpy as np

import concourse.bass as bass
import concourse.tile as tile
from concourse import mybir
from concourse.bass_utils import run_bass_kernel_spmd

F32 = mybir.dt.float32
BF16 = mybir.dt.bfloat16

NUM_BINS = 500
DOSE_MAX = 75.0
C1 = 499.0 / DOSE_MAX
GUARD = 0.4998
MAGIC = 12582912.0  # 1.5 * 2**23
JSTAR = 505  # masked voxels collapse to this bin
QSTAR, RSTAR = JSTAR // 32, JSTAR % 32
XSTAR = (JSTAR + 0.3) / C1  # maps to JSTAR with rounding margin

QW, RW, G = 16, 32, 8

# r-side slot encodings: "d" = delta (one-hot, DVE), "a" = abs (ACT)
# abs slots spread across the range for conditioning.
ACT_R_SET = (0, 2, 5, 8, 11, 14, 17, 20, 23, 26, 29)
ONES_R = 31  # constant all-ones column (cheap memset, row of ones in K)
DVE_R_SET = tuple(
    w for w in range(RW) if w not in ACT_R_SET and w != ONES_R
)

# ---- sync-wait legalization (same as baseline) ----
_ENGINE_SEM_PREFIX = {
    mybir.EngineType.DVE: "DVE_",
    mybir.EngineType.Activation: "Activation_",
    mybir.EngineType.Pool: "Pool_",
}
_EXEMPT_TYPES = (
    "InstCall",
    "InstUnconditionalBranch",
    "InstRegisterMove",
    "InstISA",
    "InstNoOp",
)
_SELF_DROP_TYPES = (
    "InstTensorTensor",
    "InstTensorScalarPtr",
    "InstTensorReduce",
    "InstActivation",
    "InstMemset",
    "InstTensorCopy",
)


def legalize_sync_waits(nc, max_waits=1):
    eng_map = {
        mybir.EngineType.DVE: nc.vector,
        mybir.EngineType.Activation: nc.scalar,
        mybir.EngineType.Pool: nc.gpsimd,
        mybir.EngineType.PE: nc.tensor,
        mybir.EngineType.SP: nc.sync,
    }
    for fn in nc.m.functions:
        blocks = list(fn.blocks)
        for blk in blocks:
            insts = blk.instructions
            work = []
            for i, ins in enumerate(insts):
                tname = type(ins).__name__
                if tname in _EXEMPT_TYPES:
                    continue
                si = ins.sync_info
                if si is None:
                    continue
                waits = list(si.on_wait)
                eng = ins.engine
                pref = _ENGINE_SEM_PREFIX.get(eng)
                if pref is not None and tname in _SELF_DROP_TYPES:
                    waits = [
                        w for w in waits
                        if not (w.ant_name or "").startswith(pref)
                    ]
                if len(waits) == len(si.on_wait) and len(waits) <= max_waits:
                    continue
                work.append((i, ins, waits))
            for i, ins, waits in reversed(work):
                si = ins.sync_info
                keep, excess = waits[:max_waits], waits[max_waits:]
                ins.sync_info = mybir.SyncInfo(
                    on_wait=keep, on_update=si.on_update
                )
                eng_iface = eng_map[ins.engine]
                for w in reversed(excess):
                    bi = eng_iface.nop(nofuse=True)
                    mi = bi.ins
                    for b2 in fn.blocks:
                        L = b2.instructions
                        for k in range(len(L) - 1, -1, -1):
                            if L[k] is mi or L[k].name == mi.name:
                                del L[k]
                                break
                        else:
                            continue
                        break
                    mi.sync_info = mybir.SyncInfo(on_wait=[w], on_update=[])
                    blk.instructions.insert(i, mi)


def build_kernel(P=128, FPP=8192, F=512, chain_chunks=1):
    assert FPP % F == 0
    nchunks = FPP // F
    NI = F // G
    nchains = nchunks // chain_chunks
    nc = bass.Bass()

    d_p_ext = nc.declare_dram_parameter("d_pred", [P, FPP], F32, isOutput=False)
    d_g_ext = nc.declare_dram_parameter("d_gt", [P, FPP], F32, isOutput=False)
    m_ext = nc.declare_dram_parameter("mask", [P, FPP], F32, isOutput=False)
    hp_ext = nc.declare_dram_parameter("hp", [P, nchains * 256], F32, isOutput=True)
    hg_ext = nc.declare_dram_parameter("hg", [P, nchains * 256], F32, isOutput=True)

    with tile.TileContext(nc) as tc, ExitStack() as ctx:
        singles = ctx.enter_context(tc.tile_pool(name="singles", bufs=1))
        ins = ctx.enter_context(tc.tile_pool(name="ins", bufs=2))
        mids = ctx.enter_context(tc.tile_pool(name="mids", bufs=2))
        hots = ctx.enter_context(tc.tile_pool(name="hots", bufs=2))
        psums = ctx.enter_context(
            tc.tile_pool(name="psums", bufs=2, space=bass.MemorySpace.PSUM)
        )

        # per-ACT-slot bias tiles (-w) for Abs encoding
        bias_t = {}
        for w in ACT_R_SET:
            bt = singles.tile([P, 1], F32, tag=f"bias{w}")
            nc.vector.memset(bt, float(-w))
            bias_t[w] = bt

        ext_map = {"p": hp_ext, "g": hg_ext}
        pending = {"p": None, "g": None}

        def flush_drain(which):
            if pending[which] is None:
                return
            prev_ps, prev_ci = pending[which]
            dr = mids.tile([P, 256], F32, tag=f"dr_{which}")
            nc.vector.tensor_copy(out=dr, in_=prev_ps)
            nc.sync.dma_start(
                out=ext_map[which][:, prev_ci * 256 : (prev_ci + 1) * 256],
                in_=dr,
            )
            pending[which] = None

        PUSH = 640.0 / C1  # masked voxels -> j ~ [640, 1140): q >= 20
        U_S1 = -PUSH
        U_S2 = PUSH - GUARD / C1

        F2 = 2 * F
        for sc in range(nchunks // 2):
            sl = slice(sc * F2, (sc + 1) * F2)
            d_p = ins.tile([P, F2], F32, tag="d_p")
            d_g = ins.tile([P, F2], F32, tag="d_g")
            m = ins.tile([P, F2], F32, tag="m")
            nc.sync.dma_start(out=m, in_=m_ext[:, sl])
            nc.sync.dma_start(out=d_p, in_=d_p_ext[:, sl])
            nc.sync.dma_start(out=d_g, in_=d_g_ext[:, sl])

            # u = m * (-PUSH) + (PUSH - GUARD/C1): pushes masked q out of
            # [0,16) so the q one-hot zeroes their contribution entirely.
            wv = mids.tile([P, F2], F32, tag="wv")
            nc.vector.tensor_scalar(
                out=wv, in0=m, scalar1=U_S1, scalar2=U_S2,
                op0=mybir.AluOpType.mult, op1=mybir.AluOpType.add,
            )
            for which, d_t in (("p", d_p), ("g", d_g)):
                # x2 = d + u
                x2 = mids.tile([P, F2], F32, tag="x2")
                nc.vector.tensor_tensor(
                    out=x2, in0=d_t, in1=wv, op=mybir.AluOpType.add
                )
                # ACT chain: t = rnd(C1*x2 + MAGIC) = MAGIC + j
                t = mids.tile([P, F2], F32, tag="t")
                nc.scalar.activation(
                    out=t, in_=x2, func=mybir.ActivationFunctionType.Copy,
                    bias=MAGIC, scale=C1,
                )
                # f2 = t/32 - 393216.484375 = j/32 - 0.484375  (exact)
                f2 = mids.tile([P, F2], F32, tag="f2")
                nc.scalar.activation(
                    out=f2, in_=t, func=mybir.ActivationFunctionType.Copy,
                    bias=-393216.484375, scale=0.03125,
                )
                # qm = rnd(f2 + MAGIC) = MAGIC + q
                qm = mids.tile([P, F2], F32, tag="qm")
                nc.scalar.activation(
                    out=qm, in_=f2, func=mybir.ActivationFunctionType.Copy,
                    bias=MAGIC, scale=1.0,
                )
                # q_bf = qm - MAGIC (bf16, exact)
                q_bf = mids.tile([P, F2], BF16, tag="q_bf")
                nc.scalar.activation(
                    out=q_bf, in_=qm, func=mybir.ActivationFunctionType.Copy,
                    bias=-MAGIC, scale=1.0,
                )
                # v = 32*qm - 32*MAGIC + MAGIC = MAGIC + 32q   (DVE)
                v = mids.tile([P, F2], F32, tag="v")
                nc.vector.tensor_scalar(
                    out=v, in0=qm, scalar1=32.0, scalar2=-390070272.0,
                    op0=mybir.AluOpType.mult, op1=mybir.AluOpType.add,
                )
                # r_bf = t - v (bf16, exact)
                r_bf = mids.tile([P, F2], BF16, tag="r_bf")
                nc.vector.tensor_tensor(
                    out=r_bf, in0=t, in1=v, op=mybir.AluOpType.subtract
                )

                for h in range(2):
                    hsl = slice(h * F, (h + 1) * F)
                    chain_idx = sc * 2 + h
                    ps = psums.tile([P, 256], F32, tag=f"ps_{which}")
                    q_bf_h = q_bf[:, hsl]
                    r_bf_h = r_bf[:, hsl]
                    q4 = q_bf_h.rearrange("p (a b) -> p a b", b=G)

                    # stationary: ah4[p, i, wq, g] = (q == wq)  (slab layout)
                    ah4 = hots.tile([P, NI, QW, G], BF16, tag="ah4")
                    for w in range(QW):
                        nc.vector.tensor_scalar(
                            out=ah4[:, :, w, :], in0=q4, scalar1=float(w),
                            scalar2=None, op0=mybir.AluOpType.is_equal,
                        )
                    # moving: bh[p, wr, f], natural layout
                    bh = hots.tile([P, RW, F], BF16, tag="bh")
                    nc.vector.memset(bh[:, ONES_R, :], 1.0)
                    for w in DVE_R_SET:
                        nc.vector.tensor_scalar(
                            out=bh[:, w, :], in0=r_bf_h, scalar1=float(w),
                            scalar2=None, op0=mybir.AluOpType.is_equal,
                        )
                    for w in ACT_R_SET:
                        nc.scalar.activation(
                            out=bh[:, w, :], in_=r_bf_h,
                            func=mybir.ActivationFunctionType.Abs,
                            bias=bias_t[w][:, 0:1], scale=1.0,
                        )

                    for i in range(NI):
                        nc.tensor.matmul(
                            ps, ah4[:, i, :, :], bh[:, :, i * G : (i + 1) * G],
                            start=(i == 0),
                            stop=(i == NI - 1),
                        )
                    flush_drain(which)
                    pending[which] = (ps, chain_idx)


        flush_drain("p")
        flush_drain("g")

    legalize_sync_waits(nc)
    return nc


NCORES = 8
P = 128
FPP = 8192
F = 512
NCHUNKS = FPP // F
CHAIN_CHUNKS = 1
NCHAINS = NCHUNKS // CHAIN_CHUNKS

_CACHE = {}


def _get_nc():
    if "nc" not in _CACHE:
        _CACHE["nc"] = build_kernel(P=P, FPP=FPP, F=F, chain_chunks=CHAIN_CHUNKS)
    return _CACHE["nc"]


def _encoding_matrix():
    """K[w, r]: delta rows for DVE slots, |r - w| rows for ACT slots."""
    K = np.zeros((RW, RW), np.float64)
    for w in range(RW):
        if w in ACT_R_SET:
            K[w, :] = np.abs(np.arange(RW) - w)
        elif w == ONES_R:
            K[w, :] = 1.0
        else:
            K[w, w] = 1.0
    return K


_KINV = np.linalg.inv(_encoding_matrix())


def _enc_vec(r):
    """Encoding response column for a voxel at r: g[w] = K[w, r]."""
    g = np.zeros(RW, np.float64)
    for w in range(RW):
        if w in ACT_R_SET:
            g[w] = abs(r - w)
        elif w == ONES_R:
            g[w] = 1.0
        else:
            g[w] = 1.0 if w == r else 0.0
    return g


def run_device(d_pred, d_gt, mask, trace=False, tmpdir=None):
    B = d_pred.shape[0]
    V = int(np.prod(d_pred.shape[1:]))
    dp = np.ascontiguousarray(d_pred, dtype=np.float32).reshape(B, V)
    dg = np.ascontiguousarray(d_gt, dtype=np.float32).reshape(B, V)
    mm = np.ascontiguousarray(mask, dtype=np.float32).reshape(B, V)
    half = V // 2
    in_maps = []
    for core in range(NCORES):
        b, h = divmod(core, 2)
        sl = slice(h * half, (h + 1) * half)
        in_maps.append(
            {
                "d_pred": dp[b, sl].reshape(P, FPP),
                "d_gt": dg[b, sl].reshape(P, FPP),
                "mask": mm[b, sl].reshape(P, FPP),
            }
        )
    res = run_bass_kernel_spmd(
        _get_nc(), in_maps, list(range(NCORES)), trace=trace, tmpdir=tmpdir
    )
    return res.results, res.exec_time_ns


def _decode_hist(hraw, n_masked):
    """hraw: [P, NCHAINS*256] f32 -> histogram [512] (float64)."""
    M = hraw.astype(np.float64).reshape(P, NCHAINS, 256).sum(axis=1)
    # M[wq*8+g, wr*8+g'] ; diagonal blocks g==g'
    M4 = M.reshape(QW, G, RW, G)
    Mfull = np.zeros((RW, QW), np.float64)
    for g in range(G):
        Mfull += M4[:, g, :, g].T
    # invert r-encoding per q
    J = _KINV @ Mfull  # [r, q]
    hist = np.zeros(512, np.float64)
    for q in range(QW):
        hist[32 * q : 32 * q + 32] = J[:, q]
    return hist


def kernel(d_pred, d_gt, mask):
    results, _ = run_device(d_pred, d_gt, mask)
    B = d_pred.shape[0]
    V = int(np.prod(d_pred.shape[1:]))
    mm = np.asarray(mask, dtype=np.float64).reshape(B, V)
    half = V // 2
    core_msums = [
        float(mm[c // 2, (c % 2) * half : (c % 2 + 1) * half].sum())
        for c in range(NCORES)
    ]
    loss = 0.0
    for b in range(B):
        e = np.zeros(512, np.float64)
        msum = 0.0
        for h in range(2):
            r = results[2 * b + h]
            core_msum = core_msums[2 * b + h]
            n_masked = P * FPP - core_msum
            hp = _decode_hist(r["hp"], n_masked)
            hg = _decode_hist(r["hg"], n_masked)
            e += hp - hg
            msum += core_msum
        ed = e[:500]
        T = np.cumsum(ed[::-1])[::-1]
        denom = msum + 1e-6
        loss += float(np.sum((T / denom) ** 2))
    loss /= B * 500
    return np.float32(loss)
